# revision 1
# baseline (speedup 1.0000x reference)
"""DCP pooling kernel for Trainium2 (8 NeuronCores, data-parallel over batch).

Math: the reference pads x spatially with zeros, takes |min over channels| of
the padded image, then sums all 3x3 sliding windows (stride 1) and finally
sums everything.  Each padded pixel is covered by cnt(h)*cnt(w) windows where
cnt is 3 in the interior and 2 at the first/last row/col (padded zero pixels
contribute nothing).  So the whole computation collapses to

    sum_{b,h,w} |min_c x[b,c,h,w]| * rw(h) * cw(w)

with rw(h) = 2 if h in {0, H-1} else 3 (same for cw).  A pure streaming
reduction: read 192 MiB, emit one scalar -> memory-bound.

Device program per core (2 images of [3,1024,1024] per core):
  for each of 16 row-tiles [128 rows x 3 channels x 1024 cols] (1.5 MB DMA):
    VectorE: channel-min via two tensor_tensor(min);
    ScalarE: absout = |m| with fused accum_out = per-row sum;
    VectorE: edge-column pair |m|[:,0] + |m|[:,W-1], then accumulate row
    sums / edge sums into a [128,6] block (all tiles / first-row-tile /
    last-row-tile variants so the host can apply the 2-vs-3 row weights).
Host: finish the weighted combine in float64 and sum the 8 cores.
"""

import numpy as np

import concourse.bass as bass
import concourse.bacc as bacc
import concourse.mybir as mybir
from concourse.alu_op_type import AluOpType
from concourse.tile import TileContext
from concourse.bass_utils import run_bass_kernel_spmd

B = 16            # full batch
NCORES = 8
BPC = B // NCORES  # batches per core
C = 3
H = W = 1024
P = 128
NT = H // P       # row-tiles per image

_CACHE: dict = {}


def build_nc(bpc: int = BPC, h: int = H, w: int = W,
             load_bufs: int = 6) -> bass.Bass:
    # Bacc (not plain Bass): its finalize() runs generate_event_semaphores,
    # which splits multi-wait instructions to satisfy the TRN2 1-wait-per-
    # instruction constraint.
    nt = h // P
    nc = bacc.Bacc()
    x = nc.declare_dram_parameter("x", [bpc, C, h, w], mybir.dt.float32,
                                  isOutput=False)
    out = nc.declare_dram_parameter("out", [P, 6], mybir.dt.float32,
                                    isOutput=True)

    with TileContext(nc) as tc:
        with (
            tc.tile_pool(name="loads", bufs=load_bufs) as loads,
            tc.tile_pool(name="work", bufs=2) as work,
            tc.tile_pool(name="small", bufs=4) as small,
            tc.tile_pool(name="accp", bufs=1) as accp,
        ):
            # acc columns: 0 = rowsum over all tiles, 1 = edgesum over all
            # tiles, 2/3 = rowsum/edgesum over t==0 tiles only (host reads
            # partition 0 = image row 0), 4/5 = over t==nt-1 tiles only
            # (host reads partition 127 = image row h-1).
            acc = accp.tile([P, 6], mybir.dt.float32)
            nc.vector.memset(acc[:], 0.0)

            # Explicit zero bias for the Abs activation, initialized on the
            # DVE so the ACT instruction's deps stay on one semaphore.
            zbias = accp.tile([P, 1], mybir.dt.float32)
            nc.vector.memset(zbias[:], 0.0)

            for b in range(bpc):
                for t in range(nt):
                    ctile = loads.tile([P, C, w], mybir.dt.float32,
                                       tag="ctile")
                    src = x[b, :, t * P:(t + 1) * P, :].rearrange(
                        "c p w -> p c w")
                    nc.sync.dma_start(out=ctile[:], in_=src)

                    t1 = work.tile([P, w], mybir.dt.float32, tag="t1")
                    nc.vector.tensor_tensor(t1[:], ctile[:, 0, :],
                                            ctile[:, 1, :], AluOpType.min)
                    m2 = work.tile([P, w], mybir.dt.float32, tag="m2")
                    nc.vector.tensor_tensor(m2[:], t1[:], ctile[:, 2, :],
                                            AluOpType.min)

                    # absout = |m2|, rowsum = sum_w |m2|  (one ACT op)
                    absout = work.tile([P, w], mybir.dt.float32, tag="absout")
                    rowsum = small.tile([P, 1], mybir.dt.float32,
                                        tag="rowsum")
                    nc.scalar.activation(absout[:], m2[:],
                                         mybir.ActivationFunctionType.Abs,
                                         bias=zbias[:],
                                         accum_out=rowsum[:])

                    # |m|(col 0) + |m|(col w-1), per row
                    edge = small.tile([P, 1], mybir.dt.float32, tag="edge")
                    nc.vector.tensor_tensor(edge[:], absout[:, 0:1],
                                            absout[:, w - 1:w],
                                            AluOpType.add)

                    nc.vector.tensor_tensor(acc[:, 0:1], acc[:, 0:1],
                                            rowsum[:], AluOpType.add)
                    nc.vector.tensor_tensor(acc[:, 1:2], acc[:, 1:2],
                                            edge[:], AluOpType.add)
                    if t == 0:
                        nc.vector.tensor_tensor(acc[:, 2:3], acc[:, 2:3],
                                                rowsum[:], AluOpType.add)
                        nc.vector.tensor_tensor(acc[:, 3:4], acc[:, 3:4],
                                                edge[:], AluOpType.add)
                    if t == nt - 1:
                        nc.vector.tensor_tensor(acc[:, 4:5], acc[:, 4:5],
                                                rowsum[:], AluOpType.add)
                        nc.vector.tensor_tensor(acc[:, 5:6], acc[:, 5:6],
                                                edge[:], AluOpType.add)

            nc.sync.dma_start(out=out[:], in_=acc[:])

    nc.finalize()
    return nc


def build_nc_raw(bpc: int = BPC, h: int = H, w: int = W,
                 nbuf: int = 8, detect_races: bool = True) -> bass.Bass:
    """Raw-Bass (no Tile) variant: hand-placed semaphores, no Tile epilogue
    barrier.  Engine programs:
      SP  : pipelined 1.5 MB HWDGE loads (nbuf slots) + final store
      DVE : channel mins + per-tile edge-column reduces + final combine
      ACT : |m| with fused per-row sum -> per-tile rowsum column
            (tiles 0..n-2; the last tile's abs+rowsum runs on the DVE so
            the tail has no cross-engine round-trip)
    Per-tile rowsum/edge values land in distinct columns; one final DVE
    combine collapses them.  The last tile's load is split (c0c1 / c2) so
    tail compute overlaps the final transfer.

    HW pitfall encoded here: a tiny DVE op must not read a location
    written by the IMMEDIATELY preceding DVE op (SBUF write-retire latency
    is exposed between back-to-back short ops and the read sees a stale
    value) -- all short-op chains below keep >=1 intervening op.  Large
    streaming ops are safe (their early elements retire long before the
    next instruction issues).
    """
    from contextlib import ExitStack

    nt = h // P
    n = bpc * nt
    assert n >= 3
    f32 = mybir.dt.float32
    # CoreSim's conservative race detector wants explicit waits even for
    # same-engine program-order deps; it is off for sim validation.
    nc = bacc.Bacc(detect_race_conditions=detect_races)
    x = nc.declare_dram_parameter("x", [bpc, C, h, w], f32, isOutput=False)
    out = nc.declare_dram_parameter("out", [P, 6], f32, isOutput=True)
    tiles = [(b, t) for b in range(bpc) for t in range(nt)]

    with ExitStack() as ctx:
        ec = ctx.enter_context
        ctiles = ec(nc.sbuf_tensor("ctiles", [P, nbuf * C * w], f32))
        t1 = ec(nc.sbuf_tensor("t1", [P, w], f32))
        m2 = ec(nc.sbuf_tensor("m2", [P, 2 * w], f32))
        ab = ec(nc.sbuf_tensor("ab", [P, 2 * w], f32))
        rowsums = ec(nc.sbuf_tensor("rowsums", [P, n], f32))
        edges0 = ec(nc.sbuf_tensor("edges0", [P, n], f32))
        edges1 = ec(nc.sbuf_tensor("edges1", [P, n], f32))
        escr = ec(nc.sbuf_tensor("escr", [P, 2], f32))
        acc = ec(nc.sbuf_tensor("acc", [P, 6], f32))
        zbias = ec(nc.sbuf_tensor("zbias", [P, 1], f32))
        acksink = ec(nc.sbuf_tensor("acksink", [P, 1], f32))
        dma_sems = [ec(nc.semaphore(f"dma_s{i}")) for i in range(nbuf)]
        last01 = ec(nc.semaphore("last01"))
        last2 = ec(nc.semaphore("last2"))
        min2_done = ec(nc.semaphore("min2_done"))
        act_done = ec(nc.semaphore("act_done"))
        fin_done = ec(nc.semaphore("fin_done"))
        out_sem = ec(nc.semaphore("out_sem"))
        block = ec(nc.Block(no_gpsimd_drain=True))

        def src_ap(b, t, c0, c1):
            return x[b, c0:c1, t * P:(t + 1) * P, :].rearrange(
                "c p w -> p c w")

        @block.sync
        def _(sync):
            for i, (b, t) in enumerate(tiles):
                if i >= nbuf:
                    # slot free once DVE consumed tile i-nbuf (min2 done);
                    # the old DMA's completion is covered transitively (DVE
                    # waited on its sem before consuming).
                    sync.wait_ge(min2_done, i - nbuf + 1)
                s = i % nbuf
                base = s * C * w
                if i < n - 1:
                    dst = ctiles[:, base:base + C * w].rearrange(
                        "p (c w) -> p c w", c=C)
                    sync.dma_start(out=dst, in_=src_ap(b, t, 0, C)
                                   ).then_inc(dma_sems[s], 16)
                else:
                    # split last load: c0c1 then c2, so tail compute starts
                    # while c2 is still in flight
                    d01 = ctiles[:, base:base + 2 * w].rearrange(
                        "p (c w) -> p c w", c=2)
                    sync.dma_start(out=d01, in_=src_ap(b, t, 0, 2)
                                   ).then_inc(last01, 16)
                    d2 = ctiles[:, base + 2 * w:base + 3 * w]
                    sync.dma_start(out=d2, in_=src_ap(b, t, 2, 3)[:, 0, :]
                                   ).then_inc(last2, 16)
            sync.wait_ge(fin_done, 1)
            sync.dma_start(out=out[:], in_=acc[:]).then_inc(out_sem, 16)
            sync.wait_ge(out_sem, 16)

        @block.vector
        def _(vector):
            vector.memset(zbias[:], 0.0)
            for i in range(n):
                s = i % nbuf
                base = s * C * w
                c0 = ctiles[:, base:base + w]
                c1 = ctiles[:, base + w:base + 2 * w]
                c2 = ctiles[:, base + 2 * w:base + 3 * w]
                ms = i % 2
                m2s = m2[:, ms * w:(ms + 1) * w]
                if i >= 2:
                    # m2 slot reuse: ACT(i-2) must have read it
                    vector.wait_ge(act_done, i - 1)
                if i < n - 1:
                    vector.wait_ge(dma_sems[s], 16 * (i // nbuf + 1))
                    vector.tensor_tensor(t1[:], c0, c1, AluOpType.min)
                else:
                    vector.wait_ge(last01, 16)
                    vector.tensor_tensor(t1[:], c0, c1, AluOpType.min)
                    vector.wait_ge(last2, 16)
                vector.tensor_tensor(m2s, t1[:], c2,
                                     AluOpType.min).then_inc(min2_done, 1)
                if i == n - 1:
                    # last tile's abs+rowsum on the DVE
                    vector.tensor_reduce(rowsums[:, i:i + 1], m2s[:],
                                         mybir.AxisListType.X, AluOpType.add,
                                         apply_absolute_value=True)
                # per-tile edge columns |m2|[:,0] and |m2|[:,w-1]
                # (two single-element reduces: strided 2-element APs misread
                # on hardware)
                vector.tensor_reduce(edges0[:, i:i + 1], m2s[:, 0:1],
                                     mybir.AxisListType.X, AluOpType.add,
                                     apply_absolute_value=True)
                vector.tensor_reduce(edges1[:, i:i + 1], m2s[:, w - 1:w],
                                     mybir.AxisListType.X, AluOpType.add,
                                     apply_absolute_value=True)

            # final combine; rowsums cols 0..n-2 are ACT's (act_done >= n-1),
            # col n-1 was just written by this engine 3 ops ago
            vector.wait_ge(act_done, n - 1)
            vector.tensor_reduce(acc[:, 0:1], rowsums[:, 0:n],
                                 mybir.AxisListType.X, AluOpType.add)
            vector.tensor_reduce(escr[:, 0:1], edges0[:, 0:n],
                                 mybir.AxisListType.X, AluOpType.add)
            vector.tensor_reduce(escr[:, 1:2], edges1[:, 0:n],
                                 mybir.AxisListType.X, AluOpType.add)
            t0_cols = [b * nt for b in range(bpc)]
            tl_cols = [b * nt + nt - 1 for b in range(bpc)]
            chains = [
                (2, [(rowsums, cc) for cc in t0_cols]),
                (4, [(rowsums, cc) for cc in tl_cols]),
                (3, [(edges0, cc) for cc in t0_cols]
                    + [(edges1, cc) for cc in t0_cols]),
                (5, [(edges0, cc) for cc in tl_cols]
                    + [(edges1, cc) for cc in tl_cols]),
            ]
            for dst, terms in chains:
                buf, cc = terms[0]
                vector.tensor_copy(acc[:, dst:dst + 1], buf[:, cc:cc + 1])
            last = vector.tensor_tensor(acc[:, 1:2], escr[:, 0:1],
                                        escr[:, 1:2], AluOpType.add)
            rounds = max(len(t) for _, t in chains) - 1
            for r in range(rounds):
                for dst, terms in chains:
                    if r + 1 < len(terms):
                        buf, cc = terms[r + 1]
                        last = vector.tensor_tensor(
                            acc[:, dst:dst + 1], acc[:, dst:dst + 1],
                            buf[:, cc:cc + 1], AluOpType.add)
            last.then_inc(fin_done, 1)

        @block.scalar
        def _(scalar):
            for i in range(n - 1):
                scalar.wait_ge(min2_done, i + 1)
                ms = i % 2
                scalar.activation(ab[:, ms * w:(ms + 1) * w],
                                  m2[:, ms * w:(ms + 1) * w],
                                  mybir.ActivationFunctionType.Abs,
                                  bias=zbias[:],
                                  accum_out=rowsums[:, i:i + 1])
                # act_done rides on a trailing copy that READS the accum
                # column: walrus splits the activation into ACTIVATE +
                # READ_ACCUMULATOR, and an inc on the activation itself can
                # fire before the accumulator lands in SBUF.
                scalar.copy(acksink[:], rowsums[:, i:i + 1]
                            ).then_inc(act_done, 1)

    nc.finalize()
    return nc


def _finish_host(results) -> np.float32:
    total = 0.0
    for r in results:
        a = np.asarray(r["out"], dtype=np.float64)
        s_all = 3.0 * a[:, 0].sum() - a[:, 1].sum()  # col-weighted total
        srow_top = 3.0 * a[0, 2] - a[0, 3]     # col-weighted sum of row 0
        srow_bot = 3.0 * a[P - 1, 4] - a[P - 1, 5]   # ... of row H-1
        total += 3.0 * s_all - srow_top - srow_bot
    return np.float32(total)


def kernel(**inputs) -> np.ndarray:
    x = np.ascontiguousarray(np.asarray(inputs["x"], dtype=np.float32))
    assert x.shape == (B, C, H, W), x.shape
    win = int(np.asarray(inputs.get("win_size", 3)))
    assert win == 3, f"kernel specialized for win_size=3, got {win}"

    if "nc" not in _CACHE:
        _CACHE["nc"] = build_nc_raw()
    nc = _CACHE["nc"]

    in_maps = [{"x": x[i * BPC:(i + 1) * BPC]} for i in range(NCORES)]
    res = run_bass_kernel_spmd(nc, in_maps, list(range(NCORES)))
    return np.array(_finish_host(res.results), dtype=np.float32)

